# revision 5
# baseline (speedup 1.0000x reference)
"""Causal self-attention (B=2, T=4096, E=768, 12 heads) on 8 TRN2 NeuronCores.

Sharding: 24 (batch, head) pairs -> 3 heads per core; cores 0-3 take batch 0,
cores 4-7 take batch 1 (heads 3c..3c+2 of that batch). Each core computes
q/k/v projections for its heads, causal flash attention, and a partial output
projection (row-slice of W_proj). Host sums the 4 partial projections per
batch and adds b_proj.

v2: globally woven schedule. The attention (j, head) pair-units form one
continuous stream; PV matmuls lag their S matmuls by 2 units (so exp latency
and ps_s buffer reuse never stall the PE), and projection matmuls (qkv of a
later block, output-projection of an earlier block) are pumped between pairs
as fillers sized to keep the PE busy while the scalar engine chews exp.
Filler plan: seg j<=3 weaves qkv(j+1); segs 4-6 add outproj(j-4); seg 7
weaves outproj(3..6); outproj(7) is the tail (3 wp1 matmuls lead it to
overlap the final recip/normalize, copies split DVE/ACT, stores split
SP/ACT HWDGE queues). Output-projection PSUM drains via DVE copies;
GPSIMD/Pool cannot touch PSUM (BIR verifier rule), and DMA cannot read
PSUM (bass rule) - drains must go through DVE or ACT.

On-device layout notes (unchanged from v1):
  - x is fed pre-transposed (xT [E, T]) so E sits on SBUF partitions.
  - Scores are computed TRANSPOSED: S^T[tk, tq] = (k @ q^T); P^T = exp(S^T)
    is directly the moving operand of the P@V matmul. No on-chip transposes.
  - Softmax denominator comes from a ones-column block appended to V.
  - No max-subtraction in softmax (scores ~N(0,1), exp safe in fp32).
  - Head-2 q and k projections share one matmul pass; W_proj is packed as
    128+64 rows so the output projection is 2 passes per E-tile.
  - Diagonal score blocks are trimmed to the causal region and packed as
    [r0|r2] (512+256) and [r1|r3] (384+128) 2-bank PSUM tiles.
"""

from collections import deque

import numpy as np
import ml_dtypes

import concourse.bass as bass
from concourse import bacc
import concourse.mybir as mybir
import concourse.tile as tile
from concourse.bass import ts
from concourse.bass_utils import run_bass_kernel_spmd

BF16 = mybir.dt.bfloat16
F32 = mybir.dt.float32
F16 = mybir.dt.float16
bf16 = ml_dtypes.bfloat16

B, T, E, NH = 2, 4096, 768, 12
D = E // NH            # 64 head dim
HPC = 3                # heads per core
KE = E // 128          # 6 contraction tiles over E
TQ = 512               # query-block (moving free dim)
NJ = T // TQ           # 8 query blocks
TK = 128               # key-block (scores partition dim)
NTK = T // TK          # 32 key blocks
TKB = 2                # key blocks per exp() batch (2 PSUM banks)
N_CORES = 8
DLEN = (512, 384, 256, 128)          # cols kept for diag block r

# filler ns available per segment (qkv unit = 5754 ns, outproj = 2556 ns)
_QKV_NS = 3 * 6 * 213 + 4 * 6 * 80
_OP_NS = 12 * 213


def _build_nc(reps=1):
    nc = bacc.Bacc()
    xT = nc.declare_dram_parameter("xT", [E, T], BF16, isOutput=False)
    # wqk columns: [ Wq heads01 (128) | Wk heads01 (128) | Wq h2 (64) | Wk h2 (64) ]
    wqk = nc.declare_dram_parameter("wqk", [E, 384], BF16, isOutput=False)
    wv = nc.declare_dram_parameter("wv", [E, HPC * D], BF16, isOutput=False)
    wp1 = nc.declare_dram_parameter("wp1", [128, E], BF16, isOutput=False)
    wp2 = nc.declare_dram_parameter("wp2", [D, E], BF16, isOutput=False)
    bqk = nc.declare_dram_parameter("bqk", [128, 3], F32, isOutput=False)
    bv = nc.declare_dram_parameter("bv", [1, HPC * D], F32, isOutput=False)
    msk = nc.declare_dram_parameter("msk", [TK, TK], BF16, isOutput=False)
    outT = nc.declare_dram_parameter("outT", [E, T], F16, isOutput=True)

    add = mybir.AluOpType.add
    scale = 1.0 / np.sqrt(D)

    with tile.TileContext(nc) as tc:
        with (
            tc.tile_pool(name="const", bufs=1) as const,
            tc.tile_pool(name="ptp", bufs=4) as ptp,
            tc.tile_pool(name="ytp", bufs=6) as ytp,
            tc.tile_pool(name="yfp", bufs=4) as yfp,
            tc.tile_pool(name="outp", bufs=6) as outp,
            tc.tile_pool(name="ps_s", bufs=2, space="PSUM") as ps_s,
            tc.tile_pool(name="ps_y", bufs=1, space="PSUM") as ps_y,
            tc.tile_pool(name="ps_a", bufs=3, space="PSUM") as ps_a,
        ):
            # ---------------- constants / activations load ----------------
            x_sb = const.tile([128, KE, T], BF16, tag="x")
            wqk_sb = const.tile([128, KE, 384], BF16, tag="wqk")
            wv_sb = const.tile([128, KE, HPC * D], BF16, tag="wv")
            wp1_sb = const.tile([128, KE, 128], BF16, tag="wp1")
            wp2_sb = const.tile([D, KE, 128], BF16, tag="wp2")
            bqk_sb = const.tile([128, 3], F32, tag="bqk")
            bv_sb = const.tile([128, HPC * D], F32, tag="bv")
            msk_sb = const.tile([TK, TK], BF16, tag="msk")

            # v tiles with 64 appended ones-columns: P@V rows 0-63 = y^T,
            # rows 64-127 = column sums of P^T (softmax denominator).
            vext = const.tile([128, HPC, NTK, 2 * D], BF16, tag="vext")

            # Startup is DMA-latency-bound. Three queues in parallel: x(0)
            # ke-tiles stream down SP HWDGE, wqk goes one-shot on the scalar
            # HWDGE queue, and bqk leads the gpsimd SWDGE queue (followed by
            # the small constants + one-shot wv).
            nc.gpsimd.dma_start(out=bqk_sb[:, :], in_=bqk[:, :])
            wqkr = wqk.rearrange("(ke p) c -> p ke c", ke=KE)
            nc.scalar.dma_start(out=wqk_sb[:, 0:3, :], in_=wqkr[:, 0:3, :])
            nc.scalar.dma_start(out=wqk_sb[:, 3:6, :], in_=wqkr[:, 3:6, :])
            for ke in range(KE):
                nc.sync.dma_start(out=x_sb[:, ke, ts(0, TQ)],
                                  in_=xT[ke * 128:(ke + 1) * 128, ts(0, TQ)])
            nc.gpsimd.dma_start(out=bv_sb[:, :], in_=bv[:, :].to_broadcast((128, HPC * D)))
            nc.gpsimd.dma_start(out=msk_sb[:, :], in_=msk[:, :])
            for ke in range(KE):
                nc.gpsimd.dma_start(out=wv_sb[:, ke, :], in_=wv[ke * 128:(ke + 1) * 128, :])
            nc.gpsimd.dma_start(
                out=wp1_sb[:, :, :],
                in_=wp1[:, :].rearrange("d (ke p) -> d ke p", ke=KE),
            )
            nc.gpsimd.dma_start(
                out=wp2_sb[:, :, :],
                in_=wp2[:, :].rearrange("d (ke p) -> d ke p", ke=KE),
            )
            for ke in range(KE):
                nc.sync.dma_start(out=x_sb[:, ke, ts(1, TQ)],
                                  in_=xT[ke * 128:(ke + 1) * 128, ts(1, TQ)])
            for j in range(2, NJ, 2):
                for ke in range(KE):
                    nc.sync.dma_start(
                        out=x_sb[:, ke, j * TQ:(j + 2) * TQ],
                        in_=xT[ke * 128:(ke + 1) * 128, j * TQ:(j + 2) * TQ])

            qT01 = const.tile([128, T], BF16, tag="qT01")
            kT01 = const.tile([128, T], BF16, tag="kT01")
            qT2 = const.tile([D, T], BF16, tag="qT2")
            kT2 = const.tile([D, T], BF16, tag="kT2")

            # "Touch" DMA-loaded constants with single-input DVE copies so the
            # DMA sync-waits attach here (2-input DVE ops have one wait slot).
            scf = const.tile([128, HPC * D], F32, tag="scf")
            scb = const.tile([TK, TK], BF16, tag="scb")
            nc.vector.tensor_copy(out=scf[:, 0:3], in_=bqk_sb[:, :])
            nc.vector.tensor_copy(out=scf[:, :], in_=bv_sb[:, :])
            nc.vector.tensor_copy(out=scb[:, :], in_=msk_sb[:, :])

            # ---------------- filler generators (PE matmul units) ----------
            def qkv_qk_gen(j, targets=((0, 0, "q01"), (128, 1, "k01"), (256, 2, "qk2"))):
                """q/k projections (transposed layouts) for block j.
                Yields approximate warm-PE ns after each matmul."""
                for (ws, bcol, dst) in targets:
                    pps = ps_a.tile([128, TQ], F32, tag="acc")
                    for ke in range(KE):
                        nc.tensor.matmul(
                            pps,
                            wqk_sb[:, ke, ws:ws + 128],
                            x_sb[:, ke, ts(j, TQ)],
                            start=(ke == 0), stop=(ke == KE - 1),
                        )
                        yield 213
                    if dst != "qk2":
                        ddst = qT01 if dst == "q01" else kT01
                        nc.vector.tensor_tensor(
                            out=ddst[:, ts(j, TQ)], in0=pps,
                            in1=bqk_sb[:, bcol:bcol + 1].to_broadcast((128, TQ)), op=add,
                        )
                    else:
                        # packed head-2 pass: rows 0:64 = q h2, 64:128 = k h2
                        nc.vector.tensor_tensor(
                            out=qT2[:, ts(j, TQ)], in0=pps[0:D, :],
                            in1=bqk_sb[0:D, 2:3].to_broadcast((D, TQ)), op=add,
                        )
                        nc.vector.tensor_tensor(
                            out=kT2[:, ts(j, TQ)], in0=pps[D:2 * D, :],
                            in1=bqk_sb[D:2 * D, 2:3].to_broadcast((D, TQ)), op=add,
                        )

            def qkv_v_gen(j):
                """v projections (+ ones columns) for block j."""
                nc.vector.memset(vext[:, :, 4 * j:4 * j + 4, D:], 1.0)
                for i in range(4 * j, 4 * j + 4):
                    vps = ps_a.tile([128, HPC * D], F32, tag="acc")
                    for ke in range(KE):
                        nc.tensor.matmul(
                            vps,
                            x_sb[:, ke, ts(i, TK)],
                            wv_sb[:, ke, :],
                            start=(ke == 0), stop=(ke == KE - 1),
                        )
                        yield 80
                    nc.vector.tensor_tensor(
                        out=vext[:, :, i, 0:D],
                        in0=vps.rearrange("p (h d) -> p h d", h=HPC),
                        in1=bv_sb.rearrange("p (h d) -> p h d", h=HPC),
                        op=add,
                    )

            def qkv_gen(j):
                yield from qkv_qk_gen(j)
                yield from qkv_v_gen(j)

            def outproj_gen(j, y01, y2):
                """Partial output projection for query block j. DVE drains the
                PSUM tile fast; stores ride the (long-idle) SP HWDGE queue."""
                for e in range(KE):
                    ops = ps_a.tile([128, TQ], F32, tag="acc")
                    nc.tensor.matmul(ops, wp1_sb[:, e, :], y01, start=True, stop=False)
                    yield 213
                    nc.tensor.matmul(ops, wp2_sb[:, e, :], y2, start=False, stop=True)
                    osb = outp.tile([128, TQ], F16, tag="o")
                    nc.vector.tensor_copy(out=osb, in_=ops)
                    nc.sync.dma_start(out=outT[ts(e, 128), ts(j, TQ)], in_=osb)
                    yield 213

            # ---------------- attention pair-units --------------------------
            def qk_slices(h, i, j, c0):
                """(k lhsT, q rhs) for head h, key block i, query cols [c0, 512)."""
                if h < 2:
                    klhs = kT01[h * D:(h + 1) * D, ts(i, TK)]
                    qrhs = qT01[h * D:(h + 1) * D, j * TQ + c0:(j + 1) * TQ]
                else:
                    klhs = kT2[:, ts(i, TK)]
                    qrhs = qT2[:, j * TQ + c0:(j + 1) * TQ]
                return klhs, qrhs

            def pair_offdiag(j, h, b0, yps):
                sps = ps_s.tile([128, TKB * TQ], F32, tag="s")
                for bi in range(TKB):
                    klhs, qrhs = qk_slices(h, b0 + bi, j, 0)
                    nc.tensor.matmul(
                        sps[:, ts(bi, TQ)], klhs, qrhs, start=True, stop=True,
                    )
                pt = ptp.tile([128, TKB * TQ], BF16, tag="pt")
                nc.scalar.activation(
                    out=pt, in_=sps,
                    func=mybir.ActivationFunctionType.Exp, scale=float(scale),
                )
                yield  # ---- PV below runs 2 units later ----
                for bi in range(TKB):
                    nc.tensor.matmul(
                        yps,
                        vext[:, h, b0 + bi, :],
                        pt[:, ts(bi, TQ)],
                        start=(b0 + bi == 0), stop=False,
                    )

            def pair_diag(j, h, ra, rb, yps, fin, fin_cols=(0, TQ)):
                """Causally trimmed diagonal blocks 4j+ra / 4j+rb. fin is the
                (y01, y2) destination pair when this unit finalizes (part of)
                the (j, h) stream: recip + normalize cols [fin_cols) after
                the PV. (Cols 0:128 are final already after the ra=0 pair.)"""
                sps = ps_s.tile([128, TKB * TQ], F32, tag="s")
                for (r, off) in ((ra, 0), (rb, DLEN[ra])):
                    klhs, qrhs = qk_slices(h, 4 * j + r, j, TK * r)
                    nc.tensor.matmul(
                        sps[:, off:off + DLEN[r]], klhs, qrhs,
                        start=True, stop=True,
                    )
                pt = ptp.tile([128, TKB * TQ], BF16, tag="pt")
                w = DLEN[ra] + DLEN[rb]
                nc.scalar.activation(
                    out=pt[:, 0:w], in_=sps[:, 0:w],
                    func=mybir.ActivationFunctionType.Exp, scale=float(scale),
                )
                for (r, off) in ((ra, 0), (rb, DLEN[ra])):
                    # intra-block triangle: first TK cols of the block
                    nc.vector.tensor_mul(
                        pt[:, off:off + TK], pt[:, off:off + TK], msk_sb[:, :],
                    )
                yield
                for (r, off) in ((ra, 0), (rb, DLEN[ra])):
                    nc.tensor.matmul(
                        yps[:, TK * r:TQ],
                        vext[:, h, 4 * j + r, :],
                        pt[:, off:off + DLEN[r]],
                        start=(j == 0 and r == 0), stop=(r == 3),
                    )
                if fin is not None:
                    y01, y2 = fin
                    c0, c1 = fin_cols
                    lr = yfp.tile([D, TQ], F32, tag="lr")
                    nc.vector.reciprocal(out=lr[:, 0:c1 - c0], in_=yps[D:2 * D, c0:c1])
                    ydst = y2 if h == 2 else y01[h * D:(h + 1) * D, :]
                    nc.vector.tensor_mul(
                        out=ydst[:, c0:c1], in0=yps[0:D, c0:c1], in1=lr[:, 0:c1 - c0])

            # ---------------- weave driver ----------------------------------
            fill = {"gen": None, "carry": 0.0}
            pend = deque()

            def pump(budget_ns):
                fill["carry"] += budget_ns
                while fill["gen"] is not None and fill["carry"] > 0:
                    try:
                        fill["carry"] -= next(fill["gen"])
                    except StopIteration:
                        fill["gen"] = None

            def drain_fill():
                while fill["gen"] is not None:
                    pump(1e9)

            def emit(unit, budget):
                next(unit)            # S matmuls + exp (+ masks)
                pend.append(unit)
                pump(budget)
                if len(pend) > 2:
                    for _ in pend.popleft():   # PV (+ finalizer)
                        pass

            def chain(gens):
                for g in gens:
                    yield from g

            for _rep in range(reps):
                # startup: q01/k01 of block 0 unwoven (ACT has nothing yet);
                # the rest of qkv(0) weaves into segment 0.
                for _ in qkv_qk_gen(0, targets=((0, 0, "q01"), (128, 1, "k01"))):
                    pass
                ys = {}
                for j in range(NJ):
                    y01 = ytp.tile([128, TQ], BF16, tag="y01")
                    y2 = ytp.tile([D, TQ], BF16, tag="y2")
                    ys[j] = (y01, y2)
                    if j == 0:
                        gens = [qkv_v_gen(0),
                                qkv_qk_gen(0, targets=((256, 2, "qk2"),)),
                                qkv_gen(1)]
                        f_ns = 4 * 6 * 80 + 6 * 213 + _QKV_NS
                    elif j <= 3:
                        gens, f_ns = [qkv_gen(j + 1)], _QKV_NS
                    elif j <= 6:
                        gens = [qkv_gen(j + 1), outproj_gen(j - 4, *ys[j - 4])]
                        f_ns = _QKV_NS + _OP_NS
                    else:
                        gens = [outproj_gen(i, *ys[i]) for i in (3, 4, 5, 6)]
                        # reserve ~1us of filler for between the final PV flush
                        # pops (covers the split recip/normalize chain)
                        f_ns = 4 * _OP_NS - 1000
                    fill["gen"] = chain(gens)
                    fill["carry"] = 0.0
                    # 0.92: slightly under-pump per pair and drain the
                    # leftover at segment end (swept 0.85-1.15; the ACT
                    # queue tolerates a late filler burst better than
                    # per-pair over-pumping)
                    budget = 0.92 * f_ns / (HPC * (2 * j + 2))
                    for h in range(HPC):
                        yps = ps_y.tile([128, TQ], F32, tag="y")
                        for b0 in range(0, 4 * j, TKB):
                            emit(pair_offdiag(j, h, b0, yps), budget)
                        emit(pair_diag(j, h, 0, 2, yps, None), budget)
                        emit(pair_diag(j, h, 1, 3, yps, ys[j]), budget)
                    if j < NJ - 1:
                        drain_fill()
                while pend:
                    for _ in pend.popleft():
                        pass
                    pump(500)
                drain_fill()
                # tail outproj(7): lead with three wp1@y01 matmuls (they only
                # need heads 0/1, so they overlap the final recip/ymul chain),
                # and spread copies/stores over two engines/queues each.
                y01, y2 = ys[7]
                tl = {}

                def _wp1(e):
                    ops = ps_a.tile([128, TQ], F32, tag="acc")
                    nc.tensor.matmul(ops, wp1_sb[:, e, :], y01, start=True, stop=False)
                    tl[e] = ops

                def _wp2(e):
                    ops = tl.pop(e)
                    nc.tensor.matmul(ops, wp2_sb[:, e, :], y2, start=False, stop=True)
                    osb = outp.tile([128, TQ], F16, tag="o")
                    # Pool/GPSIMD cannot read PSUM; ACT is idle in the tail
                    if e % 2 == 0:
                        nc.vector.tensor_copy(out=osb, in_=ops)
                    else:
                        nc.scalar.copy(out=osb, in_=ops)
                    dq = nc.sync if e % 2 == 0 else nc.scalar
                    dq.dma_start(out=outT[ts(e, 128), ts(7, TQ)], in_=osb)

                for e in range(3):
                    _wp1(e)
                for e in range(KE):
                    _wp2(e)
                    if e + 3 < KE:
                        _wp1(e + 3)
    nc.compile()
    return nc


_nc_cache = {}


def _get_nc(reps=1):
    if reps not in _nc_cache:
        _nc_cache[reps] = _build_nc(reps)
    return _nc_cache[reps]


def _make_mask():
    p = np.arange(TK)[:, None]
    c = np.arange(TK)[None, :]
    return (p <= c).astype(bf16)


def _prep_in_maps(inputs):
    x = np.asarray(inputs["x"], np.float32)
    Wa = np.asarray(inputs["W_attn"], np.float32)
    ba = np.asarray(inputs["b_attn"], np.float32)
    Wp = np.asarray(inputs["W_proj"], np.float32)
    msk = _make_mask()
    in_maps = []
    for c in range(N_CORES):
        b = c // 4
        h0 = (c % 4) * HPC * D  # column offset of this core's heads
        sl = slice(h0, h0 + HPC * D)
        Wq = Wa[:, h0:h0 + HPC * D]
        Wk = Wa[:, E + h0:E + h0 + HPC * D]
        wqk = np.concatenate(
            [Wq[:, 0:128], Wk[:, 0:128], Wq[:, 128:192], Wk[:, 128:192]], axis=1)
        bq = ba[h0:h0 + HPC * D]
        bk = ba[E + h0:E + h0 + HPC * D]
        bqk = np.stack(
            [bq[0:128], bk[0:128], np.concatenate([bq[128:192], bk[128:192]])],
            axis=1).astype(np.float32)
        Wpc = Wp[sl, :]
        in_maps.append({
            "xT": np.ascontiguousarray(x[b].T).astype(bf16),
            "wqk": np.ascontiguousarray(wqk).astype(bf16),
            "wv": np.ascontiguousarray(Wa[:, 2 * E + h0:2 * E + h0 + HPC * D]).astype(bf16),
            "wp1": np.ascontiguousarray(Wpc[0:128, :]).astype(bf16),
            "wp2": np.ascontiguousarray(Wpc[128:192, :]).astype(bf16),
            "bqk": bqk,
            "bv": ba[2 * E + h0:2 * E + h0 + HPC * D].reshape(1, HPC * D).astype(np.float32),
            "msk": msk,
        })
    return in_maps


def _run(inputs, trace=False):
    nc = _get_nc()
    in_maps = _prep_in_maps(inputs)
    res = run_bass_kernel_spmd(nc, in_maps, core_ids=list(range(N_CORES)), trace=trace)
    bp = np.asarray(inputs["b_proj"], np.float32)
    y = np.empty((B, T, E), np.float32)
    for b in range(B):
        s = res.results[4 * b]["outT"].astype(np.float32)
        for cc in range(4 * b + 1, 4 * b + 4):
            s = s + res.results[cc]["outT"].astype(np.float32)
        y[b] = s.T
    y += bp
    return y, res


def kernel(**inputs):
    return _run(inputs)[0]


# revision 6
# speedup vs baseline: 1.0017x; 1.0017x over previous
"""Causal self-attention (B=2, T=4096, E=768, 12 heads) on 8 TRN2 NeuronCores.

Sharding: 24 (batch, head) pairs -> 3 heads per core; cores 0-3 take batch 0,
cores 4-7 take batch 1 (heads 3c..3c+2 of that batch). Each core computes
q/k/v projections for its heads, causal flash attention, and a partial output
projection (row-slice of W_proj). Host sums the 4 partial projections per
batch and adds b_proj.

v2: globally woven schedule. The attention (j, head) pair-units form one
continuous stream; PV matmuls lag their S matmuls by 2 units (so exp latency
and ps_s buffer reuse never stall the PE), and projection matmuls (qkv of a
later block, output-projection of an earlier block) are pumped between pairs
as fillers sized to keep the PE busy while the scalar engine chews exp.
Filler plan: seg j<=3 weaves qkv(j+1); segs 4-6 add outproj(j-4); seg 7
weaves outproj(3..6); outproj(7) is the tail (3 wp1 matmuls lead it to
overlap the final recip/normalize, copies split DVE/ACT, stores split
SP/ACT HWDGE queues). Output-projection PSUM drains via DVE copies;
GPSIMD/Pool cannot touch PSUM (BIR verifier rule), and DMA cannot read
PSUM (bass rule) - drains must go through DVE or ACT.

On-device layout notes (unchanged from v1):
  - x is fed pre-transposed (xT [E, T]) so E sits on SBUF partitions.
  - Scores are computed TRANSPOSED: S^T[tk, tq] = (k @ q^T); P^T = exp(S^T)
    is directly the moving operand of the P@V matmul. No on-chip transposes.
  - Softmax denominator comes from a ones-column block appended to V.
  - No max-subtraction in softmax (scores ~N(0,1), exp safe in fp32).
  - Head-2 q and k projections share one matmul pass; W_proj is packed as
    128+64 rows so the output projection is 2 passes per E-tile.
  - Diagonal score blocks are trimmed to the causal region and packed as
    [r0|r2] (512+256) and [r1|r3] (384+128) 2-bank PSUM tiles.
"""

from collections import deque

import numpy as np
import ml_dtypes

import concourse.bass as bass
from concourse import bacc
import concourse.mybir as mybir
import concourse.tile as tile
from concourse.bass import ts
from concourse.bass_utils import run_bass_kernel_spmd

BF16 = mybir.dt.bfloat16
F32 = mybir.dt.float32
F16 = mybir.dt.float16
bf16 = ml_dtypes.bfloat16

B, T, E, NH = 2, 4096, 768, 12
D = E // NH            # 64 head dim
HPC = 3                # heads per core
KE = E // 128          # 6 contraction tiles over E
TQ = 512               # query-block (moving free dim)
NJ = T // TQ           # 8 query blocks
TK = 128               # key-block (scores partition dim)
NTK = T // TK          # 32 key blocks
TKB = 2                # key blocks per exp() batch (2 PSUM banks)
N_CORES = 8
DLEN = (512, 384, 256, 128)          # cols kept for diag block r

# filler ns available per segment (qkv unit = 5754 ns, outproj = 2556 ns)
_QKV_NS = 3 * 6 * 213 + 4 * 6 * 80
_OP_NS = 12 * 213


def _build_nc(reps=1):
    nc = bacc.Bacc()
    xT = nc.declare_dram_parameter("xT", [E, T], BF16, isOutput=False)
    # wqk columns: [ Wq heads01 (128) | Wk heads01 (128) | Wq h2 (64) | Wk h2 (64) ]
    wqk = nc.declare_dram_parameter("wqk", [E, 384], BF16, isOutput=False)
    wv = nc.declare_dram_parameter("wv", [E, HPC * D], BF16, isOutput=False)
    wp1 = nc.declare_dram_parameter("wp1", [128, E], BF16, isOutput=False)
    wp2 = nc.declare_dram_parameter("wp2", [D, E], BF16, isOutput=False)
    bqk = nc.declare_dram_parameter("bqk", [128, 3], F32, isOutput=False)
    bv = nc.declare_dram_parameter("bv", [1, HPC * D], F32, isOutput=False)
    msk = nc.declare_dram_parameter("msk", [TK, TK], BF16, isOutput=False)
    outT = nc.declare_dram_parameter("outT", [E, T], F16, isOutput=True)

    add = mybir.AluOpType.add
    scale = 1.0 / np.sqrt(D)

    with tile.TileContext(nc) as tc:
        with (
            tc.tile_pool(name="const", bufs=1) as const,
            tc.tile_pool(name="ptp", bufs=6) as ptp,
            tc.tile_pool(name="ytp", bufs=6) as ytp,
            tc.tile_pool(name="yfp", bufs=4) as yfp,
            tc.tile_pool(name="outp", bufs=6) as outp,
            tc.tile_pool(name="ps_s", bufs=2, space="PSUM") as ps_s,
            tc.tile_pool(name="ps_y", bufs=1, space="PSUM") as ps_y,
            tc.tile_pool(name="ps_a", bufs=3, space="PSUM") as ps_a,
        ):
            # ---------------- constants / activations load ----------------
            x_sb = const.tile([128, KE, T], BF16, tag="x")
            wqk_sb = const.tile([128, KE, 384], BF16, tag="wqk")
            wv_sb = const.tile([128, KE, HPC * D], BF16, tag="wv")
            wp1_sb = const.tile([128, KE, 128], BF16, tag="wp1")
            wp2_sb = const.tile([D, KE, 128], BF16, tag="wp2")
            bqk_sb = const.tile([128, 3], F32, tag="bqk")
            bv_sb = const.tile([128, HPC * D], F32, tag="bv")
            msk_sb = const.tile([TK, TK], BF16, tag="msk")

            # v tiles with 64 appended ones-columns: P@V rows 0-63 = y^T,
            # rows 64-127 = column sums of P^T (softmax denominator).
            vext = const.tile([128, HPC, NTK, 2 * D], BF16, tag="vext")

            # Startup is DMA-latency-bound. Three queues in parallel: x(0)
            # ke-tiles stream down SP HWDGE, wqk goes one-shot on the scalar
            # HWDGE queue, and bqk leads the gpsimd SWDGE queue (followed by
            # the small constants + one-shot wv).
            nc.gpsimd.dma_start(out=bqk_sb[:, :], in_=bqk[:, :])
            wqkr = wqk.rearrange("(ke p) c -> p ke c", ke=KE)
            nc.scalar.dma_start(out=wqk_sb[:, 0:3, :], in_=wqkr[:, 0:3, :])
            nc.scalar.dma_start(out=wqk_sb[:, 3:6, :], in_=wqkr[:, 3:6, :])
            for ke in range(KE):
                nc.sync.dma_start(out=x_sb[:, ke, ts(0, TQ)],
                                  in_=xT[ke * 128:(ke + 1) * 128, ts(0, TQ)])
            nc.gpsimd.dma_start(out=bv_sb[:, :], in_=bv[:, :].to_broadcast((128, HPC * D)))
            nc.gpsimd.dma_start(out=msk_sb[:, :], in_=msk[:, :])
            for ke in range(KE):
                nc.gpsimd.dma_start(out=wv_sb[:, ke, :], in_=wv[ke * 128:(ke + 1) * 128, :])
            nc.gpsimd.dma_start(
                out=wp1_sb[:, :, :],
                in_=wp1[:, :].rearrange("d (ke p) -> d ke p", ke=KE),
            )
            nc.gpsimd.dma_start(
                out=wp2_sb[:, :, :],
                in_=wp2[:, :].rearrange("d (ke p) -> d ke p", ke=KE),
            )
            for ke in range(KE):
                nc.sync.dma_start(out=x_sb[:, ke, ts(1, TQ)],
                                  in_=xT[ke * 128:(ke + 1) * 128, ts(1, TQ)])
            for j in range(2, NJ, 2):
                for ke in range(KE):
                    nc.sync.dma_start(
                        out=x_sb[:, ke, j * TQ:(j + 2) * TQ],
                        in_=xT[ke * 128:(ke + 1) * 128, j * TQ:(j + 2) * TQ])

            qT01 = const.tile([128, T], BF16, tag="qT01")
            kT01 = const.tile([128, T], BF16, tag="kT01")
            qT2 = const.tile([D, T], BF16, tag="qT2")
            kT2 = const.tile([D, T], BF16, tag="kT2")

            # "Touch" DMA-loaded constants with single-input DVE copies so the
            # DMA sync-waits attach here (2-input DVE ops have one wait slot).
            scf = const.tile([128, HPC * D], F32, tag="scf")
            scb = const.tile([TK, TK], BF16, tag="scb")
            nc.vector.tensor_copy(out=scf[:, 0:3], in_=bqk_sb[:, :])
            nc.vector.tensor_copy(out=scf[:, :], in_=bv_sb[:, :])
            nc.vector.tensor_copy(out=scb[:, :], in_=msk_sb[:, :])

            # ---------------- filler generators (PE matmul units) ----------
            def qkv_qk_gen(j, targets=((0, 0, "q01"), (128, 1, "k01"), (256, 2, "qk2"))):
                """q/k projections (transposed layouts) for block j.
                Yields approximate warm-PE ns after each matmul."""
                for (ws, bcol, dst) in targets:
                    pps = ps_a.tile([128, TQ], F32, tag="acc")
                    for ke in range(KE):
                        nc.tensor.matmul(
                            pps,
                            wqk_sb[:, ke, ws:ws + 128],
                            x_sb[:, ke, ts(j, TQ)],
                            start=(ke == 0), stop=(ke == KE - 1),
                        )
                        yield 213
                    if dst != "qk2":
                        ddst = qT01 if dst == "q01" else kT01
                        nc.vector.tensor_tensor(
                            out=ddst[:, ts(j, TQ)], in0=pps,
                            in1=bqk_sb[:, bcol:bcol + 1].to_broadcast((128, TQ)), op=add,
                        )
                    else:
                        # packed head-2 pass: rows 0:64 = q h2, 64:128 = k h2
                        nc.vector.tensor_tensor(
                            out=qT2[:, ts(j, TQ)], in0=pps[0:D, :],
                            in1=bqk_sb[0:D, 2:3].to_broadcast((D, TQ)), op=add,
                        )
                        nc.vector.tensor_tensor(
                            out=kT2[:, ts(j, TQ)], in0=pps[D:2 * D, :],
                            in1=bqk_sb[D:2 * D, 2:3].to_broadcast((D, TQ)), op=add,
                        )

            def qkv_v_gen(j):
                """v projections (+ ones columns) for block j."""
                nc.vector.memset(vext[:, :, 4 * j:4 * j + 4, D:], 1.0)
                for i in range(4 * j, 4 * j + 4):
                    vps = ps_a.tile([128, HPC * D], F32, tag="acc")
                    for ke in range(KE):
                        nc.tensor.matmul(
                            vps,
                            x_sb[:, ke, ts(i, TK)],
                            wv_sb[:, ke, :],
                            start=(ke == 0), stop=(ke == KE - 1),
                        )
                        yield 80
                    nc.vector.tensor_tensor(
                        out=vext[:, :, i, 0:D],
                        in0=vps.rearrange("p (h d) -> p h d", h=HPC),
                        in1=bv_sb.rearrange("p (h d) -> p h d", h=HPC),
                        op=add,
                    )

            def qkv_gen(j):
                yield from qkv_qk_gen(j)
                yield from qkv_v_gen(j)

            def outproj_gen(j, y01, y2):
                """Partial output projection for query block j. DVE drains the
                PSUM tile fast; stores ride the (long-idle) SP HWDGE queue."""
                for e in range(KE):
                    ops = ps_a.tile([128, TQ], F32, tag="acc")
                    nc.tensor.matmul(ops, wp1_sb[:, e, :], y01, start=True, stop=False)
                    yield 213
                    nc.tensor.matmul(ops, wp2_sb[:, e, :], y2, start=False, stop=True)
                    osb = outp.tile([128, TQ], F16, tag="o")
                    nc.vector.tensor_copy(out=osb, in_=ops)
                    nc.sync.dma_start(out=outT[ts(e, 128), ts(j, TQ)], in_=osb)
                    yield 213

            # ---------------- attention pair-units --------------------------
            def qk_slices(h, i, j, c0):
                """(k lhsT, q rhs) for head h, key block i, query cols [c0, 512)."""
                if h < 2:
                    klhs = kT01[h * D:(h + 1) * D, ts(i, TK)]
                    qrhs = qT01[h * D:(h + 1) * D, j * TQ + c0:(j + 1) * TQ]
                else:
                    klhs = kT2[:, ts(i, TK)]
                    qrhs = qT2[:, j * TQ + c0:(j + 1) * TQ]
                return klhs, qrhs

            def pair_offdiag(j, h, b0, yps):
                sps = ps_s.tile([128, TKB * TQ], F32, tag="s")
                for bi in range(TKB):
                    klhs, qrhs = qk_slices(h, b0 + bi, j, 0)
                    nc.tensor.matmul(
                        sps[:, ts(bi, TQ)], klhs, qrhs, start=True, stop=True,
                    )
                pt = ptp.tile([128, TKB * TQ], BF16, tag="pt")
                nc.scalar.activation(
                    out=pt, in_=sps,
                    func=mybir.ActivationFunctionType.Exp, scale=float(scale),
                )
                yield  # ---- PV below runs 2 units later ----
                for bi in range(TKB):
                    nc.tensor.matmul(
                        yps,
                        vext[:, h, b0 + bi, :],
                        pt[:, ts(bi, TQ)],
                        start=(b0 + bi == 0), stop=False,
                    )

            def pair_diag(j, h, ra, rb, yps, fin, fin_cols=(0, TQ)):
                """Causally trimmed diagonal blocks 4j+ra / 4j+rb. fin is the
                (y01, y2) destination pair when this unit finalizes (part of)
                the (j, h) stream: recip + normalize cols [fin_cols) after
                the PV. (Cols 0:128 are final already after the ra=0 pair.)"""
                sps = ps_s.tile([128, TKB * TQ], F32, tag="s")
                for (r, off) in ((ra, 0), (rb, DLEN[ra])):
                    klhs, qrhs = qk_slices(h, 4 * j + r, j, TK * r)
                    nc.tensor.matmul(
                        sps[:, off:off + DLEN[r]], klhs, qrhs,
                        start=True, stop=True,
                    )
                pt = ptp.tile([128, TKB * TQ], BF16, tag="pt")
                w = DLEN[ra] + DLEN[rb]
                nc.scalar.activation(
                    out=pt[:, 0:w], in_=sps[:, 0:w],
                    func=mybir.ActivationFunctionType.Exp, scale=float(scale),
                )
                for (r, off) in ((ra, 0), (rb, DLEN[ra])):
                    # intra-block triangle: first TK cols of the block
                    nc.vector.tensor_mul(
                        pt[:, off:off + TK], pt[:, off:off + TK], msk_sb[:, :],
                    )
                yield
                for (r, off) in ((ra, 0), (rb, DLEN[ra])):
                    nc.tensor.matmul(
                        yps[:, TK * r:TQ],
                        vext[:, h, 4 * j + r, :],
                        pt[:, off:off + DLEN[r]],
                        start=(j == 0 and r == 0), stop=(r == 3),
                    )
                if fin is not None:
                    y01, y2 = fin
                    c0, c1 = fin_cols
                    lr = yfp.tile([D, TQ], F32, tag="lr")
                    nc.vector.reciprocal(out=lr[:, 0:c1 - c0], in_=yps[D:2 * D, c0:c1])
                    ydst = y2 if h == 2 else y01[h * D:(h + 1) * D, :]
                    nc.vector.tensor_mul(
                        out=ydst[:, c0:c1], in0=yps[0:D, c0:c1], in1=lr[:, 0:c1 - c0])

            # ---------------- weave driver ----------------------------------
            fill = {"gen": None, "carry": 0.0}
            pend = deque()

            def pump(budget_ns):
                fill["carry"] += budget_ns
                while fill["gen"] is not None and fill["carry"] > 0:
                    try:
                        fill["carry"] -= next(fill["gen"])
                    except StopIteration:
                        fill["gen"] = None

            def drain_fill():
                while fill["gen"] is not None:
                    pump(1e9)

            # PV lag L=3 (pop at >3): one more parked unit than the
            # minimum lets PE ride through ACT hiccups (L=2: +0.8us,
            # L=4: head-handoff stalls; ptp bufs=6 holds the extra pt)
            def emit(unit, budget):
                next(unit)            # S matmuls + exp (+ masks)
                pend.append(unit)
                pump(budget)
                if len(pend) > 3:
                    for _ in pend.popleft():   # PV (+ finalizer)
                        pass

            def chain(gens):
                for g in gens:
                    yield from g

            for _rep in range(reps):
                # startup: q01/k01 of block 0 unwoven (ACT has nothing yet);
                # the rest of qkv(0) weaves into segment 0.
                for _ in qkv_qk_gen(0, targets=((0, 0, "q01"), (128, 1, "k01"))):
                    pass
                ys = {}
                for j in range(NJ):
                    y01 = ytp.tile([128, TQ], BF16, tag="y01")
                    y2 = ytp.tile([D, TQ], BF16, tag="y2")
                    ys[j] = (y01, y2)
                    if j == 0:
                        gens = [qkv_v_gen(0),
                                qkv_qk_gen(0, targets=((256, 2, "qk2"),)),
                                qkv_gen(1)]
                        f_ns = 4 * 6 * 80 + 6 * 213 + _QKV_NS
                    elif j <= 3:
                        gens, f_ns = [qkv_gen(j + 1)], _QKV_NS
                    elif j <= 6:
                        gens = [qkv_gen(j + 1), outproj_gen(j - 4, *ys[j - 4])]
                        f_ns = _QKV_NS + _OP_NS
                    else:
                        gens = [outproj_gen(i, *ys[i]) for i in (3, 4, 5, 6)]
                        # reserve ~1us of filler for between the final PV flush
                        # pops (covers the split recip/normalize chain)
                        f_ns = 4 * _OP_NS - 1000
                    fill["gen"] = chain(gens)
                    fill["carry"] = 0.0
                    # 0.92: slightly under-pump per pair and drain the
                    # leftover at segment end (swept 0.85-1.15; the ACT
                    # queue tolerates a late filler burst better than
                    # per-pair over-pumping). Seg 7 consumes everything
                    # in-stream: post-flush leftovers would block on DVE
                    # behind the final recip/normalize chain.
                    m = 0.92
                    budget = m * f_ns / (HPC * (2 * j + 2))
                    for h in range(HPC):
                        yps = ps_y.tile([128, TQ], F32, tag="y")
                        for b0 in range(0, 4 * j, TKB):
                            emit(pair_offdiag(j, h, b0, yps), budget)
                        emit(pair_diag(j, h, 0, 2, yps, None), budget)
                        last = (j == NJ - 1 and h == HPC - 1)
                        emit(pair_diag(j, h, 1, 3, yps,
                                       None if last else ys[j]), budget)
                        if last:
                            last_yps = yps
                    if j < NJ - 1:
                        drain_fill()
                while pend:
                    for _ in pend.popleft():
                        pass
                    pump(500)
                drain_fill()
                # tail outproj(7): lead with three wp1@y01 matmuls (they only
                # need heads 0/1, so they overlap the final recip/ymul chain),
                # and spread copies/stores over two engines/queues each.
                y01, y2 = ys[7]
                tl = {}

                def _wp1(e):
                    ops = ps_a.tile([128, TQ], F32, tag="acc")
                    nc.tensor.matmul(ops, wp1_sb[:, e, :], y01, start=True, stop=False)
                    tl[e] = ops

                def _wp2(e):
                    ops = tl.pop(e)
                    nc.tensor.matmul(ops, wp2_sb[:, e, :], y2, start=False, stop=True)
                    osb = outp.tile([128, TQ], F16, tag="o")
                    # Pool/GPSIMD cannot read PSUM; ACT is idle in the tail
                    if e % 2 == 0:
                        nc.vector.tensor_copy(out=osb, in_=ops)
                    else:
                        nc.scalar.copy(out=osb, in_=ops)
                    dq = nc.sync if e % 2 == 0 else nc.scalar
                    dq.dma_start(out=outT[ts(e, 128), ts(7, TQ)], in_=osb)

                for e in range(3):
                    _wp1(e)
                # manual finalizer for (7, h2), emitted AFTER the wp1 lead:
                # a matmul's single cross-engine wait slot coarsens to the
                # latest DVE instruction at emission time, so leads emitted
                # after recip/ymul would stall behind the whole chain
                lr = yfp.tile([D, TQ], F32, tag="lr")
                nc.vector.reciprocal(out=lr, in_=last_yps[D:2 * D, :])
                nc.vector.tensor_mul(out=y2, in0=last_yps[0:D, :], in1=lr)
                for e in range(KE):
                    _wp2(e)
                    if e + 3 < KE:
                        _wp1(e + 3)
    nc.compile()
    return nc


_nc_cache = {}


def _get_nc(reps=1):
    if reps not in _nc_cache:
        _nc_cache[reps] = _build_nc(reps)
    return _nc_cache[reps]


def _make_mask():
    p = np.arange(TK)[:, None]
    c = np.arange(TK)[None, :]
    return (p <= c).astype(bf16)


def _prep_in_maps(inputs):
    x = np.asarray(inputs["x"], np.float32)
    Wa = np.asarray(inputs["W_attn"], np.float32)
    ba = np.asarray(inputs["b_attn"], np.float32)
    Wp = np.asarray(inputs["W_proj"], np.float32)
    msk = _make_mask()
    in_maps = []
    for c in range(N_CORES):
        b = c // 4
        h0 = (c % 4) * HPC * D  # column offset of this core's heads
        sl = slice(h0, h0 + HPC * D)
        Wq = Wa[:, h0:h0 + HPC * D]
        Wk = Wa[:, E + h0:E + h0 + HPC * D]
        wqk = np.concatenate(
            [Wq[:, 0:128], Wk[:, 0:128], Wq[:, 128:192], Wk[:, 128:192]], axis=1)
        bq = ba[h0:h0 + HPC * D]
        bk = ba[E + h0:E + h0 + HPC * D]
        bqk = np.stack(
            [bq[0:128], bk[0:128], np.concatenate([bq[128:192], bk[128:192]])],
            axis=1).astype(np.float32)
        Wpc = Wp[sl, :]
        in_maps.append({
            "xT": np.ascontiguousarray(x[b].T).astype(bf16),
            "wqk": np.ascontiguousarray(wqk).astype(bf16),
            "wv": np.ascontiguousarray(Wa[:, 2 * E + h0:2 * E + h0 + HPC * D]).astype(bf16),
            "wp1": np.ascontiguousarray(Wpc[0:128, :]).astype(bf16),
            "wp2": np.ascontiguousarray(Wpc[128:192, :]).astype(bf16),
            "bqk": bqk,
            "bv": ba[2 * E + h0:2 * E + h0 + HPC * D].reshape(1, HPC * D).astype(np.float32),
            "msk": msk,
        })
    return in_maps


def _run(inputs, trace=False):
    nc = _get_nc()
    in_maps = _prep_in_maps(inputs)
    res = run_bass_kernel_spmd(nc, in_maps, core_ids=list(range(N_CORES)), trace=trace)
    bp = np.asarray(inputs["b_proj"], np.float32)
    y = np.empty((B, T, E), np.float32)
    for b in range(B):
        s = res.results[4 * b]["outT"].astype(np.float32)
        for cc in range(4 * b + 1, 4 * b + 4):
            s = s + res.results[cc]["outT"].astype(np.float32)
        y[b] = s.T
    y += bp
    return y, res


def kernel(**inputs):
    return _run(inputs)[0]


# revision 7
# speedup vs baseline: 1.0063x; 1.0045x over previous
"""Causal self-attention (B=2, T=4096, E=768, 12 heads) on 8 TRN2 NeuronCores.

Sharding: 24 (batch, head) pairs -> 3 heads per core; cores 0-3 take batch 0,
cores 4-7 take batch 1 (heads 3c..3c+2 of that batch). Each core computes
q/k/v projections for its heads, causal flash attention, and a partial output
projection (row-slice of W_proj). Host sums the 4 partial projections per
batch and adds b_proj.

v2: globally woven schedule. The attention (j, head) pair-units form one
continuous stream; PV matmuls lag their S matmuls by 2 units (so exp latency
and ps_s buffer reuse never stall the PE), and projection matmuls (qkv of a
later block, output-projection of an earlier block) are pumped between pairs
as fillers sized to keep the PE busy while the scalar engine chews exp.
Filler plan: seg j<=3 weaves qkv(j+1); segs 4-6 add outproj(j-4); seg 7
weaves outproj(3..6); outproj(7) is the tail (3 wp1 matmuls lead it to
overlap the final recip/normalize, copies split DVE/ACT, stores split
SP/ACT HWDGE queues). Output-projection PSUM drains via DVE copies;
GPSIMD/Pool cannot touch PSUM (BIR verifier rule), and DMA cannot read
PSUM (bass rule) - drains must go through DVE or ACT.

On-device layout notes (unchanged from v1):
  - x is fed pre-transposed (xT [E, T]) so E sits on SBUF partitions.
  - Scores are computed TRANSPOSED: S^T[tk, tq] = (k @ q^T); P^T = exp(S^T)
    is directly the moving operand of the P@V matmul. No on-chip transposes.
  - Softmax denominator comes from a ones-column block appended to V.
  - No max-subtraction in softmax (scores ~N(0,1), exp safe in fp32).
  - Head-2 q and k projections share one matmul pass; W_proj is packed as
    128+64 rows so the output projection is 2 passes per E-tile.
  - Diagonal score blocks are trimmed to the causal region and packed as
    [r0|r2] (512+256) and [r1|r3] (384+128) 2-bank PSUM tiles.
"""

from collections import deque

import numpy as np
import ml_dtypes

import concourse.bass as bass
from concourse import bacc
import concourse.mybir as mybir
import concourse.tile as tile
from concourse.bass import ts
from concourse.bass_utils import run_bass_kernel_spmd

BF16 = mybir.dt.bfloat16
F32 = mybir.dt.float32
F16 = mybir.dt.float16
bf16 = ml_dtypes.bfloat16

B, T, E, NH = 2, 4096, 768, 12
D = E // NH            # 64 head dim
HPC = 3                # heads per core
KE = E // 128          # 6 contraction tiles over E
TQ = 512               # query-block (moving free dim)
NJ = T // TQ           # 8 query blocks
TK = 128               # key-block (scores partition dim)
NTK = T // TK          # 32 key blocks
TKB = 2                # key blocks per exp() batch (2 PSUM banks)
N_CORES = 8
DLEN = (512, 384, 256, 128)          # cols kept for diag block r

# filler ns available per segment (qkv unit = 5754 ns, outproj = 2556 ns)
_QKV_NS = 3 * 6 * 213 + 4 * 6 * 80
_OP_NS = 12 * 213


def _build_nc(reps=1):
    nc = bacc.Bacc()
    xT = nc.declare_dram_parameter("xT", [E, T], BF16, isOutput=False)
    # wqk columns: [ Wq heads01 (128) | Wk heads01 (128) | Wq h2 (64) | Wk h2 (64) ]
    wqk = nc.declare_dram_parameter("wqk", [E, 384], BF16, isOutput=False)
    wv = nc.declare_dram_parameter("wv", [E, HPC * D], BF16, isOutput=False)
    wp1 = nc.declare_dram_parameter("wp1", [128, E], BF16, isOutput=False)
    wp2 = nc.declare_dram_parameter("wp2", [D, E], BF16, isOutput=False)
    bqk = nc.declare_dram_parameter("bqk", [128, 3], F32, isOutput=False)
    bv = nc.declare_dram_parameter("bv", [1, HPC * D], F32, isOutput=False)
    msk = nc.declare_dram_parameter("msk", [TK, TK], BF16, isOutput=False)
    outT = nc.declare_dram_parameter("outT", [E, T], F16, isOutput=True)

    add = mybir.AluOpType.add
    scale = 1.0 / np.sqrt(D)

    with tile.TileContext(nc) as tc:
        with (
            tc.tile_pool(name="const", bufs=1) as const,
            tc.tile_pool(name="ptp", bufs=6) as ptp,
            tc.tile_pool(name="ytp", bufs=6) as ytp,
            tc.tile_pool(name="yfp", bufs=4) as yfp,
            tc.tile_pool(name="outp", bufs=6) as outp,
            tc.tile_pool(name="ps_s", bufs=2, space="PSUM") as ps_s,
            tc.tile_pool(name="ps_y", bufs=1, space="PSUM") as ps_y,
            tc.tile_pool(name="ps_a", bufs=3, space="PSUM") as ps_a,
        ):
            # ---------------- constants / activations load ----------------
            x_sb = const.tile([128, KE, T], BF16, tag="x")
            wqk_sb = const.tile([128, KE, 384], BF16, tag="wqk")
            wv_sb = const.tile([128, KE, HPC * D], BF16, tag="wv")
            wp1_sb = const.tile([128, KE, 128], BF16, tag="wp1")
            wp2_sb = const.tile([D, KE, 128], BF16, tag="wp2")
            bqk_sb = const.tile([128, 3], F32, tag="bqk")
            bv_sb = const.tile([128, HPC * D], F32, tag="bv")
            msk_sb = const.tile([TK, TK], BF16, tag="msk")

            # v tiles with 64 appended ones-columns: P@V rows 0-63 = y^T,
            # rows 64-127 = column sums of P^T (softmax denominator).
            vext = const.tile([128, HPC, NTK, 2 * D], BF16, tag="vext")

            # Startup is DMA-latency-bound. Three queues in parallel: x(0)
            # ke-tiles stream down SP HWDGE, wqk goes one-shot on the scalar
            # HWDGE queue, and bqk leads the gpsimd SWDGE queue (followed by
            # the small constants + one-shot wv).
            nc.gpsimd.dma_start(out=bqk_sb[:, :], in_=bqk[:, :])
            wqkr = wqk.rearrange("(ke p) c -> p ke c", ke=KE)
            nc.scalar.dma_start(out=wqk_sb[:, 0:3, :], in_=wqkr[:, 0:3, :])
            nc.scalar.dma_start(out=wqk_sb[:, 3:6, :], in_=wqkr[:, 3:6, :])
            for ke in range(KE):
                nc.sync.dma_start(out=x_sb[:, ke, ts(0, TQ)],
                                  in_=xT[ke * 128:(ke + 1) * 128, ts(0, TQ)])
            nc.gpsimd.dma_start(out=bv_sb[:, :], in_=bv[:, :].to_broadcast((128, HPC * D)))
            nc.gpsimd.dma_start(out=msk_sb[:, :], in_=msk[:, :])
            for ke in range(KE):
                nc.gpsimd.dma_start(out=wv_sb[:, ke, :], in_=wv[ke * 128:(ke + 1) * 128, :])
            nc.gpsimd.dma_start(
                out=wp1_sb[:, :, :],
                in_=wp1[:, :].rearrange("d (ke p) -> d ke p", ke=KE),
            )
            nc.gpsimd.dma_start(
                out=wp2_sb[:, :, :],
                in_=wp2[:, :].rearrange("d (ke p) -> d ke p", ke=KE),
            )
            for ke in range(KE):
                nc.sync.dma_start(out=x_sb[:, ke, ts(1, TQ)],
                                  in_=xT[ke * 128:(ke + 1) * 128, ts(1, TQ)])
            for j in range(2, NJ, 2):
                for ke in range(KE):
                    nc.sync.dma_start(
                        out=x_sb[:, ke, j * TQ:(j + 2) * TQ],
                        in_=xT[ke * 128:(ke + 1) * 128, j * TQ:(j + 2) * TQ])

            qT01 = const.tile([128, T], BF16, tag="qT01")
            kT01 = const.tile([128, T], BF16, tag="kT01")
            qT2 = const.tile([D, T], BF16, tag="qT2")
            kT2 = const.tile([D, T], BF16, tag="kT2")

            # "Touch" DMA-loaded constants with single-input DVE copies so the
            # DMA sync-waits attach here (2-input DVE ops have one wait slot).
            scf = const.tile([128, HPC * D], F32, tag="scf")
            scb = const.tile([TK, TK], BF16, tag="scb")
            nc.vector.tensor_copy(out=scf[:, 0:3], in_=bqk_sb[:, :])
            nc.vector.tensor_copy(out=scf[:, :], in_=bv_sb[:, :])
            nc.vector.tensor_copy(out=scb[:, :], in_=msk_sb[:, :])

            # ---------------- filler generators (PE matmul units) ----------
            def qkv_qk_gen(j, targets=((0, 0, "q01"), (128, 1, "k01"), (256, 2, "qk2"))):
                """q/k projections (transposed layouts) for block j.
                Yields approximate warm-PE ns after each matmul."""
                for (ws, bcol, dst) in targets:
                    pps = ps_a.tile([128, TQ], F32, tag="acc")
                    for ke in range(KE):
                        nc.tensor.matmul(
                            pps,
                            wqk_sb[:, ke, ws:ws + 128],
                            x_sb[:, ke, ts(j, TQ)],
                            start=(ke == 0), stop=(ke == KE - 1),
                        )
                        yield 213
                    if dst != "qk2":
                        ddst = qT01 if dst == "q01" else kT01
                        nc.vector.tensor_tensor(
                            out=ddst[:, ts(j, TQ)], in0=pps,
                            in1=bqk_sb[:, bcol:bcol + 1].to_broadcast((128, TQ)), op=add,
                        )
                    else:
                        # packed head-2 pass: rows 0:64 = q h2, 64:128 = k h2
                        nc.vector.tensor_tensor(
                            out=qT2[:, ts(j, TQ)], in0=pps[0:D, :],
                            in1=bqk_sb[0:D, 2:3].to_broadcast((D, TQ)), op=add,
                        )
                        nc.vector.tensor_tensor(
                            out=kT2[:, ts(j, TQ)], in0=pps[D:2 * D, :],
                            in1=bqk_sb[D:2 * D, 2:3].to_broadcast((D, TQ)), op=add,
                        )

            def qkv_v_gen(j):
                """v projections (+ ones columns) for block j."""
                nc.vector.memset(vext[:, :, 4 * j:4 * j + 4, D:], 1.0)
                for i in range(4 * j, 4 * j + 4):
                    vps = ps_a.tile([128, HPC * D], F32, tag="acc")
                    for ke in range(KE):
                        nc.tensor.matmul(
                            vps,
                            x_sb[:, ke, ts(i, TK)],
                            wv_sb[:, ke, :],
                            start=(ke == 0), stop=(ke == KE - 1),
                        )
                        yield 80
                    nc.vector.tensor_tensor(
                        out=vext[:, :, i, 0:D],
                        in0=vps.rearrange("p (h d) -> p h d", h=HPC),
                        in1=bv_sb.rearrange("p (h d) -> p h d", h=HPC),
                        op=add,
                    )

            def qkv_gen(j):
                yield from qkv_qk_gen(j)
                yield from qkv_v_gen(j)

            def outproj_gen(j, y01, y2):
                """Partial output projection for query block j. DVE drains the
                PSUM tile fast; stores ride the (long-idle) SP HWDGE queue."""
                for e in range(KE):
                    ops = ps_a.tile([128, TQ], F32, tag="acc")
                    nc.tensor.matmul(ops, wp1_sb[:, e, :], y01, start=True, stop=False)
                    yield 213
                    nc.tensor.matmul(ops, wp2_sb[:, e, :], y2, start=False, stop=True)
                    osb = outp.tile([128, TQ], F16, tag="o")
                    nc.vector.tensor_copy(out=osb, in_=ops)
                    nc.sync.dma_start(out=outT[ts(e, 128), ts(j, TQ)], in_=osb)
                    yield 213

            # ---------------- attention pair-units --------------------------
            def qk_slices(h, i, j, c0):
                """(k lhsT, q rhs) for head h, key block i, query cols [c0, 512)."""
                if h < 2:
                    klhs = kT01[h * D:(h + 1) * D, ts(i, TK)]
                    qrhs = qT01[h * D:(h + 1) * D, j * TQ + c0:(j + 1) * TQ]
                else:
                    klhs = kT2[:, ts(i, TK)]
                    qrhs = qT2[:, j * TQ + c0:(j + 1) * TQ]
                return klhs, qrhs

            def pair_offdiag(j, h, b0, yps):
                sps = ps_s.tile([128, TKB * TQ], F32, tag="s")
                for bi in range(TKB):
                    klhs, qrhs = qk_slices(h, b0 + bi, j, 0)
                    nc.tensor.matmul(
                        sps[:, ts(bi, TQ)], klhs, qrhs, start=True, stop=True,
                    )
                pt = ptp.tile([128, TKB * TQ], BF16, tag="pt")
                nc.scalar.activation(
                    out=pt, in_=sps,
                    func=mybir.ActivationFunctionType.Exp, scale=float(scale),
                )
                yield  # ---- PV below runs 2 units later ----
                for bi in range(TKB):
                    nc.tensor.matmul(
                        yps,
                        vext[:, h, b0 + bi, :],
                        pt[:, ts(bi, TQ)],
                        start=(b0 + bi == 0), stop=False,
                    )

            def pair_diag(j, h, ra, rb, yps, fin, fin_cols=(0, TQ)):
                """Causally trimmed diagonal blocks 4j+ra / 4j+rb. fin is the
                (y01, y2) destination pair when this unit finalizes (part of)
                the (j, h) stream: recip + normalize cols [fin_cols) after
                the PV. (Cols 0:128 are final already after the ra=0 pair.)"""
                sps = ps_s.tile([128, TKB * TQ], F32, tag="s")
                for (r, off) in ((ra, 0), (rb, DLEN[ra])):
                    klhs, qrhs = qk_slices(h, 4 * j + r, j, TK * r)
                    nc.tensor.matmul(
                        sps[:, off:off + DLEN[r]], klhs, qrhs,
                        start=True, stop=True,
                    )
                pt = ptp.tile([128, TKB * TQ], BF16, tag="pt")
                w = DLEN[ra] + DLEN[rb]
                nc.scalar.activation(
                    out=pt[:, 0:w], in_=sps[:, 0:w],
                    func=mybir.ActivationFunctionType.Exp, scale=float(scale),
                )
                for (r, off) in ((ra, 0), (rb, DLEN[ra])):
                    # intra-block triangle: first TK cols of the block
                    nc.vector.tensor_mul(
                        pt[:, off:off + TK], pt[:, off:off + TK], msk_sb[:, :],
                    )
                yield
                for (r, off) in ((ra, 0), (rb, DLEN[ra])):
                    nc.tensor.matmul(
                        yps[:, TK * r:TQ],
                        vext[:, h, 4 * j + r, :],
                        pt[:, off:off + DLEN[r]],
                        start=(j == 0 and r == 0), stop=(r == 3),
                    )
                if fin is not None:
                    y01, y2 = fin
                    c0, c1 = fin_cols
                    lr = yfp.tile([D, TQ], F32, tag="lr")
                    nc.vector.reciprocal(out=lr[:, 0:c1 - c0], in_=yps[D:2 * D, c0:c1])
                    ydst = y2 if h == 2 else y01[h * D:(h + 1) * D, :]
                    nc.vector.tensor_mul(
                        out=ydst[:, c0:c1], in0=yps[0:D, c0:c1], in1=lr[:, 0:c1 - c0])

            # ---------------- weave driver ----------------------------------
            fill = {"gen": None, "carry": 0.0}
            pend = deque()

            def pump(budget_ns):
                fill["carry"] += budget_ns
                while fill["gen"] is not None and fill["carry"] > 0:
                    try:
                        fill["carry"] -= next(fill["gen"])
                    except StopIteration:
                        fill["gen"] = None

            def drain_fill():
                while fill["gen"] is not None:
                    pump(1e9)

            # PV lag L=3 (pop at >3): one more parked unit than the
            # minimum lets PE ride through ACT hiccups (L=2: +0.8us,
            # L=4: head-handoff stalls; ptp bufs=6 holds the extra pt).
            # Fillers pump AFTER the pop: period order [S, PV, fillers]
            # measured 256ns better than [S, fillers, PV].
            def emit(unit, budget):
                next(unit)            # S matmuls + exp (+ masks)
                pend.append(unit)
                if len(pend) > 3:
                    for _ in pend.popleft():   # PV (+ finalizer)
                        pass
                pump(budget)

            def chain(gens):
                for g in gens:
                    yield from g

            for _rep in range(reps):
                # startup: q01/k01 of block 0 unwoven (ACT has nothing yet);
                # the rest of qkv(0) weaves into segment 0.
                for _ in qkv_qk_gen(0, targets=((0, 0, "q01"), (128, 1, "k01"))):
                    pass
                ys = {}
                for j in range(NJ):
                    y01 = ytp.tile([128, TQ], BF16, tag="y01")
                    y2 = ytp.tile([D, TQ], BF16, tag="y2")
                    ys[j] = (y01, y2)
                    if j == 0:
                        gens = [qkv_v_gen(0),
                                qkv_qk_gen(0, targets=((256, 2, "qk2"),)),
                                qkv_gen(1)]
                        f_ns = 4 * 6 * 80 + 6 * 213 + _QKV_NS
                    elif j <= 3:
                        gens, f_ns = [qkv_gen(j + 1)], _QKV_NS
                    elif j <= 6:
                        gens = [qkv_gen(j + 1), outproj_gen(j - 4, *ys[j - 4])]
                        f_ns = _QKV_NS + _OP_NS
                    else:
                        gens = [outproj_gen(i, *ys[i]) for i in (3, 4, 5, 6)]
                        # reserve ~1us of filler for between the final PV flush
                        # pops (covers the split recip/normalize chain)
                        f_ns = 4 * _OP_NS - 1000
                    fill["gen"] = chain(gens)
                    fill["carry"] = 0.0
                    # 0.92: slightly under-pump per pair and drain the
                    # leftover at segment end (swept 0.85-1.15; the ACT
                    # queue tolerates a late filler burst better than
                    # per-pair over-pumping). Seg 7 consumes everything
                    # in-stream: post-flush leftovers would block on DVE
                    # behind the final recip/normalize chain.
                    m = 0.92
                    budget = m * f_ns / (HPC * (2 * j + 2))
                    for h in range(HPC):
                        yps = ps_y.tile([128, TQ], F32, tag="y")
                        for b0 in range(0, 4 * j, TKB):
                            emit(pair_offdiag(j, h, b0, yps), budget)
                        emit(pair_diag(j, h, 0, 2, yps, None), budget)
                        last = (j == NJ - 1 and h == HPC - 1)
                        emit(pair_diag(j, h, 1, 3, yps,
                                       None if last else ys[j]), budget)
                        if last:
                            last_yps = yps
                    if j < NJ - 1:
                        drain_fill()
                while pend:
                    for _ in pend.popleft():
                        pass
                    pump(500)
                drain_fill()
                # tail outproj(7): lead with three wp1@y01 matmuls (they only
                # need heads 0/1, so they overlap the final recip/ymul chain),
                # and spread copies/stores over two engines/queues each.
                y01, y2 = ys[7]
                tl = {}

                def _wp1(e):
                    ops = ps_a.tile([128, TQ], F32, tag="acc")
                    nc.tensor.matmul(ops, wp1_sb[:, e, :], y01, start=True, stop=False)
                    tl[e] = ops

                def _wp2(e):
                    ops = tl.pop(e)
                    nc.tensor.matmul(ops, wp2_sb[:, e, :], y2, start=False, stop=True)
                    osb = outp.tile([128, TQ], F16, tag="o")
                    # Pool/GPSIMD cannot read PSUM; ACT is idle in the tail
                    if e % 2 == 0:
                        nc.vector.tensor_copy(out=osb, in_=ops)
                    else:
                        nc.scalar.copy(out=osb, in_=ops)
                    dq = nc.sync if e % 2 == 0 else nc.scalar
                    dq.dma_start(out=outT[ts(e, 128), ts(7, TQ)], in_=osb)

                for e in range(3):
                    _wp1(e)
                # manual finalizer for (7, h2), emitted AFTER the wp1 lead:
                # a matmul's single cross-engine wait slot coarsens to the
                # latest DVE instruction at emission time, so leads emitted
                # after recip/ymul would stall behind the whole chain
                lr = yfp.tile([D, TQ], F32, tag="lr")
                nc.vector.reciprocal(out=lr, in_=last_yps[D:2 * D, :])
                nc.vector.tensor_mul(out=y2, in0=last_yps[0:D, :], in1=lr)
                for e in range(KE):
                    _wp2(e)
                    if e + 3 < KE:
                        _wp1(e + 3)
    nc.compile()
    return nc


_nc_cache = {}


def _get_nc(reps=1):
    if reps not in _nc_cache:
        _nc_cache[reps] = _build_nc(reps)
    return _nc_cache[reps]


def _make_mask():
    p = np.arange(TK)[:, None]
    c = np.arange(TK)[None, :]
    return (p <= c).astype(bf16)


def _prep_in_maps(inputs):
    x = np.asarray(inputs["x"], np.float32)
    Wa = np.asarray(inputs["W_attn"], np.float32)
    ba = np.asarray(inputs["b_attn"], np.float32)
    Wp = np.asarray(inputs["W_proj"], np.float32)
    msk = _make_mask()
    in_maps = []
    for c in range(N_CORES):
        b = c // 4
        h0 = (c % 4) * HPC * D  # column offset of this core's heads
        sl = slice(h0, h0 + HPC * D)
        Wq = Wa[:, h0:h0 + HPC * D]
        Wk = Wa[:, E + h0:E + h0 + HPC * D]
        wqk = np.concatenate(
            [Wq[:, 0:128], Wk[:, 0:128], Wq[:, 128:192], Wk[:, 128:192]], axis=1)
        bq = ba[h0:h0 + HPC * D]
        bk = ba[E + h0:E + h0 + HPC * D]
        bqk = np.stack(
            [bq[0:128], bk[0:128], np.concatenate([bq[128:192], bk[128:192]])],
            axis=1).astype(np.float32)
        Wpc = Wp[sl, :]
        in_maps.append({
            "xT": np.ascontiguousarray(x[b].T).astype(bf16),
            "wqk": np.ascontiguousarray(wqk).astype(bf16),
            "wv": np.ascontiguousarray(Wa[:, 2 * E + h0:2 * E + h0 + HPC * D]).astype(bf16),
            "wp1": np.ascontiguousarray(Wpc[0:128, :]).astype(bf16),
            "wp2": np.ascontiguousarray(Wpc[128:192, :]).astype(bf16),
            "bqk": bqk,
            "bv": ba[2 * E + h0:2 * E + h0 + HPC * D].reshape(1, HPC * D).astype(np.float32),
            "msk": msk,
        })
    return in_maps


def _run(inputs, trace=False):
    nc = _get_nc()
    in_maps = _prep_in_maps(inputs)
    res = run_bass_kernel_spmd(nc, in_maps, core_ids=list(range(N_CORES)), trace=trace)
    bp = np.asarray(inputs["b_proj"], np.float32)
    y = np.empty((B, T, E), np.float32)
    for b in range(B):
        s = res.results[4 * b]["outT"].astype(np.float32)
        for cc in range(4 * b + 1, 4 * b + 4):
            s = s + res.results[cc]["outT"].astype(np.float32)
        y[b] = s.T
    y += bp
    return y, res


def kernel(**inputs):
    return _run(inputs)[0]
